# revision 16
# baseline (speedup 1.0000x reference)
"""Trainium2 Bass kernel for a 2-layer GAT block (gnn_message_passing).

Strategy (8 NeuronCores, dst-node sharding, fp16 tables, dma_gather):
  - Nodes padded to 50176 = 8*6272; core m owns rows [6272m, 6272(m+1)).
    49 groups of 128 own dst nodes per core.
  - Layer-1 node transform is REPLICATED (x is a full input on every core):
    each core computes the whole table1[50176, 256] = fp16(x @ W1) in HBM.
    No collective for layer 1.  A small per-core pass computes a_dst1-dots
    and the residual x@Wfc for own nodes only.
  - Edge phase: edges sharded by dst; per 128-dst group the src rows are
    fetched with ONE dma_gather per (group, table-half) (int16 indices cap
    rows at 32768, so tables are split at row 25088).  Per 128-edge block a
    one-hot S matrix turns segment-sum and a_dst-expansion into PE matmuls;
    a_src-dots are recomputed on-chip from the gathered rows (keeps table
    rows at 512B).  ex = exp(leaky_relu(as+ad)) batched per group.
  - f1 evacuation transposes own f1 into SBUF; layer-2 node transform for
    own nodes feeds table2 chunks that are AllGathered (7 chunks of 7
    groups) OVERLAPPED with the remaining layer-1 edge work.
  - Edge phase 2 gathers from the AllGathered table2 (chunked row layout,
    indices precomputed on host), evacuates mean-over-heads + residual.
"""

import numpy as np

import concourse.bass as bass
import concourse.bacc as bacc
import concourse.mybir as mybir
import concourse.tile as tile
from concourse.bass_utils import run_bass_kernel_spmd

# Problem constants (hardcoded per harness contract)
N = 50000
E = 800000
IN_C = 128
OUT_C = 64
HEADS = 4
NEG_SLOPE = 0.2
N_CORES = 8

P = 128
NPC = 6272                 # own nodes per core (padded); 8*6272 = 50176
G = NPC // P               # 49 own groups per core
NPAD = N_CORES * NPC       # 50176
GALL = NPAD // P           # 392 groups in the replicated layer-1 transform
SPLIT = NPAD // 2          # 25088: table half split (int16 gather indices)
# table2 is split into 3 sub-tables (AllGathered as soon as their groups
# are evacuated, overlapping remaining edge-1 work).  Own-group ranges:
G2SPLITS = (16, 32)        # sub-table a: groups [0,16), b: [16,32), c: [32,49)
T2SIZES = (N_CORES * 16 * P, N_CORES * 16 * P, N_CORES * 17 * P)
T2BASES = (0, T2SIZES[0], T2SIZES[0] + T2SIZES[1])
HC = HEADS * OUT_C         # 256
W1COLS = HC + HEADS + OUT_C  # 324: W1 | a_dst-dot | Wfc
W2COLS = HC + HEADS          # 260: W2 | a_dst-dot

FP32 = mybir.dt.float32
FP16 = mybir.dt.float16
I16 = mybir.dt.int16

# timing-triage mode (set by triage.py): None | "noedge" | "gather" |
# "noag" | "nogather" | "nosblock"
TRIAGE = None


def _ceil_div(a, b):
    return (a + b - 1) // b


# ---------------------------------------------------------------------------
# Host-side preprocessing
# ---------------------------------------------------------------------------

def _row2_of_src(src):
    """Row of node `src` in the three-part table2 layout ([m, gg, r]-major
    within each sub-table; sub-table bases offset the combined index)."""
    m = src // NPC
    loc = src % NPC
    gg = loc >> 7
    r = loc & 127
    a = 2048 * m + 128 * gg + r
    b = T2BASES[1] + 2048 * m + 128 * (gg - 16) + r
    c = T2BASES[2] + 2176 * m + 128 * (gg - 32) + r
    return np.where(gg < 16, a, np.where(gg < 32, b, c))


def _preprocess(edge_index):
    """Sort/shard/pad edges; per-layer gather indices + dst-local arrays with
    a block schedule that is uniform across cores (SPMD: one program)."""
    src = np.asarray(edge_index[0], dtype=np.int64)
    dst = np.asarray(edge_index[1], dtype=np.int64)
    loops = np.arange(N, dtype=np.int64)
    src = np.concatenate([src, loops]).astype(np.int64)
    dst = np.concatenate([dst, loops]).astype(np.int64)

    core = dst // NPC
    # table1 is partition-major: node (g, r) = (src>>7, src&127) sits at
    # row r*GALL + g, making phase-A table writes contiguous per partition.
    row1 = (src & 127) * GALL + (src >> 7)
    row2 = _row2_of_src(src)

    LBASES = {1: [0, SPLIT], 2: list(T2BASES)}

    percore = []   # per core: dict(layer -> (rows, key, dloc))
    cnts = {1: [], 2: []}   # per core: [G, nparts] counts
    for m in range(N_CORES):
        mask = core == m
        cs = src[mask]
        r1 = row1[mask]
        cd = dst[mask] - m * NPC
        gg = cd >> 7
        dl = cd & 127
        r2 = row2[mask]
        layers = {}
        for l, rows in ((1, r1), (2, r2)):
            bases = LBASES[l]
            npart = len(bases)
            part = np.searchsorted(bases[1:], rows, side="right")
            o = np.lexsort((rows, part, gg))
            lr = rows[o]
            lp = part[o]
            lg = gg[o]
            ld = dl[o]
            cnt = np.zeros((G, npart), dtype=np.int64)
            np.add.at(cnt, (lg, lp), 1)
            layers[l] = (lr, lg * npart + lp, ld)
            cnts[l].append(cnt)
        percore.append(layers)

    scheds = {}
    for l in (1, 2):
        allc = np.stack(cnts[l])                  # [cores, G, nparts]
        nbp = np.maximum(_ceil_div(allc, P).max(axis=0), 1)  # [G, nparts]
        btot = int(nbp.sum())
        scheds[l] = dict(NBP=nbp, BTOT=btot, MAXB=int(nbp.sum(axis=1).max()),
                         BASES=LBASES[l])

    # Pair-merged block schedule: groups are processed in pairs (2k, 2k+1);
    # within a pair, blocks are ordered part-major ((g0,h0),(g1,h0),(g0,h1),
    # ...) so ONE dma_gather per (pair, part) covers both groups.  PAIRS[k]
    # holds per-part (col_start, nblocks) and per-group block-slot lists.
    for l in (1, 2):
        nbp = scheds[l]["NBP"]
        npart = nbp.shape[1]
        pairs = []
        t = 0
        for k in range(0, G, 2):
            gs = [k] if k + 1 >= G else [k, k + 1]
            parts = []
            slots = {g: [] for g in gs}
            start = t
            for h in range(npart):
                cs = t
                for g in gs:
                    nbh = int(nbp[g, h])
                    slots[g].extend(range(t - start, t - start + nbh))
                    t += nbh
                parts.append((cs, t - cs))
            pairs.append(dict(start=start, parts=parts,
                              groups=[(g, slots[g]) for g in gs],
                              total=t - start))
        scheds[l]["PAIRS"] = pairs
        scheds[l]["PMAXB"] = max(p["total"] for p in pairs)
        assert t == scheds[l]["BTOT"]

    # per-core padded arrays (same pair-merged order)
    coredata = []
    for m in range(N_CORES):
        out = {}
        for l in (1, 2):
            nbp = scheds[l]["NBP"]
            bases = scheds[l]["BASES"]
            npart = len(bases)
            btot = scheds[l]["BTOT"]
            rows, key, dloc = percore[m][l]
            order_bounds = np.searchsorted(key, np.arange(npart * G + 1))
            idxw = np.zeros((128, 8 * btot), dtype=np.int16)
            dl_arr = np.full((128, btot), -1.0, dtype=np.float32)
            t = 0
            for k in range(0, G, 2):
                gs = [k] if k + 1 >= G else [k, k + 1]
                for h in range(npart):
                    for g in gs:
                        nbh = int(nbp[g, h])
                        a, b = (order_bounds[npart * g + h],
                                order_bounds[npart * g + h + 1])
                        ne = b - a
                        npadd = nbh * P - ne
                        assert npadd >= 0
                        rr = np.concatenate([
                            rows[a:b] - bases[h],
                            np.zeros(npadd, np.int64)]).astype(np.int16)
                        dd = np.concatenate([
                            dloc[a:b].astype(np.float32),
                            np.full(npadd, -1.0, np.float32)])
                        nn = nbh * P
                        iw = np.zeros((16, nn // 16), np.int16)
                        iw[np.arange(nn) % 16, np.arange(nn) // 16] = rr
                        idxw[:, 8 * t: 8 * (t + nbh)] = np.tile(iw, (8, 1))
                        dl_arr[np.arange(nn) % 128,
                               t + np.arange(nn) // 128] = dd
                        t += nbh
            assert t == btot
            out[f"idx{l}"] = idxw
            out[f"dl{l}"] = dl_arr
        coredata.append(out)
    return scheds, coredata


# ---------------------------------------------------------------------------
# Device program
# ---------------------------------------------------------------------------

def _build_program(scheds, add_b1, reps=1):
    nc = bacc.Bacc(
        "TRN2",
        target_bir_lowering=False,
        debug=False,
        enable_asserts=False,
        num_devices=N_CORES,
        num_swdge_queues=4,
    )

    B1, B2 = scheds[1]["BTOT"], scheds[2]["BTOT"]

    # ---- I/O ----
    xTfull_d = nc.dram_tensor("xTfull", [IN_C, NPAD], FP16, kind="ExternalInput")
    xTown_d = nc.dram_tensor("xTown", [IN_C, NPC], FP16, kind="ExternalInput")
    idx1_d = nc.dram_tensor("idx1", [128, 8 * B1], I16, kind="ExternalInput")
    idx2_d = nc.dram_tensor("idx2", [128, 8 * B2], I16, kind="ExternalInput")
    dl1_d = nc.dram_tensor("dl1", [128, B1], FP32, kind="ExternalInput")
    dl2_d = nc.dram_tensor("dl2", [128, B2], FP32, kind="ExternalInput")
    w1ext_d = nc.dram_tensor("w1ext", [IN_C, W1COLS], FP16, kind="ExternalInput")
    w2ext_d = nc.dram_tensor("w2ext", [HC, W2COLS], FP16, kind="ExternalInput")
    asrcb1_d = nc.dram_tensor("asrcb1", [P, HC], FP16, kind="ExternalInput")
    asrcb2_d = nc.dram_tensor("asrcb2", [P, HC], FP16, kind="ExternalInput")
    iota_d = nc.dram_tensor("iota", [P, P], FP32, kind="ExternalInput")
    ident_d = nc.dram_tensor("ident", [P, P], FP16, kind="ExternalInput")
    if add_b1:
        b1rep_d = nc.dram_tensor("b1rep", [P, HC], FP32, kind="ExternalInput")
    out_d = nc.dram_tensor("out", [NPC, OUT_C], FP32, kind="ExternalOutput")

    with tile.TileContext(nc) as tc:
        with (
            tc.tile_pool(name="const", bufs=1) as cpool,
            tc.tile_pool(name="dram", bufs=1, space="DRAM") as dpool,
        ):
            iota_t = cpool.tile([P, P], FP32)
            nc.sync.dma_start(out=iota_t[:], in_=iota_d[:])
            ident_t = cpool.tile([P, P], FP16)
            nc.sync.dma_start(out=ident_t[:], in_=ident_d[:])
            w1_t = cpool.tile([IN_C, W1COLS], FP16)
            nc.sync.dma_start(out=w1_t[:], in_=w1ext_d[:])
            w2a_t = cpool.tile([P, W2COLS], FP16)
            nc.sync.dma_start(out=w2a_t[:], in_=w2ext_d[0:P, :])
            w2b_t = cpool.tile([P, W2COLS], FP16)
            nc.sync.dma_start(out=w2b_t[:], in_=w2ext_d[P: 2 * P, :])
            asrcb1_t = cpool.tile([P, HC], FP16)
            nc.sync.dma_start(out=asrcb1_t[:], in_=asrcb1_d[:])
            asrcb2_t = cpool.tile([P, HC], FP16)
            nc.sync.dma_start(out=asrcb2_t[:], in_=asrcb2_d[:])
            idx1_t = cpool.tile([128, 8 * B1], I16)
            nc.sync.dma_start(out=idx1_t[:], in_=idx1_d[:])
            idx2_t = cpool.tile([128, 8 * B2], I16)
            nc.sync.dma_start(out=idx2_t[:], in_=idx2_d[:])
            dl1_t = cpool.tile([128, B1], FP32)
            nc.sync.dma_start(out=dl1_t[:], in_=dl1_d[:])
            dl2_t = cpool.tile([128, B2], FP32)
            nc.sync.dma_start(out=dl2_t[:], in_=dl2_d[:])
            if add_b1:
                b1_t = cpool.tile([P, HC], FP32)
                nc.sync.dma_start(out=b1_t[:], in_=b1rep_d[:])

            for rep in range(reps):
              with tc.tile_pool(name=f"state{rep}", bufs=1) as statepool:
                table1 = dpool.tile([NPAD, HC], FP16, tag=f"t1_{rep}",
                                    name=f"table1_{rep}")
                t2own = dpool.tile([NPC, HC], FP16, tag=f"t2o_{rep}",
                                   name=f"t2own_{rep}")
                t2tiles = [
                    dpool.tile([T2SIZES[i], HC], FP16, addr_space="Shared",
                               tag=f"t2{i}_{rep}", name=f"table2{i}_{rep}")
                    for i in range(3)
                ]

                alde1_sb = statepool.tile([P, G * HEADS], FP16, tag="ad1")
                alde2_sb = statepool.tile([P, G * HEADS], FP16, tag="ad2")
                xch_sb = statepool.tile([P, G * OUT_C], FP32, tag="xch")
                f1T_sb = statepool.tile([P, G * HC], FP16, tag="f1T")

                # ---------- Phase A-own: a_dst1-dots + residual (own nodes) --
                with (
                    tc.tile_pool(name=f"po{rep}", bufs=3) as po,
                    tc.tile_pool(name=f"po_ps{rep}", bufs=2, space="PSUM") as po_ps,
                ):
                    xo = po.tile([IN_C, NPC], FP16, tag="xo")
                    nc.sync.dma_start(out=xo[:], in_=xTown_d[:])
                    for g in range(G):
                        ps = po_ps.tile([P, HEADS + OUT_C], FP32, tag="ps")
                        nc.tensor.matmul(
                            ps[:], lhsT=xo[:, g * P:(g + 1) * P],
                            rhs=w1_t[:, HC:W1COLS],
                            start=True, stop=True)
                        nc.vector.tensor_copy(
                            alde1_sb[:, g * HEADS:(g + 1) * HEADS],
                            ps[:, 0:HEADS])
                        nc.vector.tensor_copy(
                            xch_sb[:, g * OUT_C:(g + 1) * OUT_C],
                            ps[:, HEADS:HEADS + OUT_C])

                # ---------- Phase A-full: replicated layer-1 transform -------
                # super-groups of SG groups: one big x read + one big table
                # write per super-group (batched DMA).
                SG = 8
                with (
                    tc.tile_pool(name=f"pa{rep}", bufs=2) as pa,
                    tc.tile_pool(name=f"pa_ps{rep}", bufs=4, space="PSUM") as pa_ps,
                ):
                    for g0 in range(0, GALL, SG):
                        ng = min(SG, GALL - g0)
                        xf = pa.tile([IN_C, SG * P], FP16, tag="xf")
                        nc.sync.dma_start(
                            out=xf[:, 0:ng * P],
                            in_=xTfull_d[:, g0 * P:(g0 + ng) * P])
                        tx = pa.tile([P, SG * HC], FP16, tag="tx")
                        for k in range(ng):
                            ph = pa_ps.tile([P, HC], FP32, tag="ph")
                            nc.tensor.matmul(
                                ph[:], lhsT=xf[:, k * P:(k + 1) * P],
                                rhs=w1_t[:, 0:HC], start=True, stop=True)
                            nc.scalar.copy(
                                tx[:, k * HC:(k + 1) * HC], ph[:])
                        outap = bass.AP(
                            table1[:].tensor,
                            table1[:].offset + g0 * HC,
                            [[GALL * HC, P], [1, ng * HC]])
                        nc.sync.dma_start(out=outap, in_=tx[:, 0:ng * HC])

                # ---------- Edge phase ----------
                def edge_phase(layer, sched, tabs, idx_t, dl_t,
                               alde_sb, asrcb_t, evac):
                    NBP = sched["NBP"]
                    MAXB = sched["MAXB"]
                    npart = NBP.shape[1]
                    with (
                        tc.tile_pool(name=f"gt{layer}", bufs=2) as gpool,
                        tc.tile_pool(name=f"mz{layer}", bufs=2) as mzpool,
                        tc.tile_pool(name=f"sS{layer}",
                                     bufs=sched["PMAXB"] + MAXB + 2) as spool,
                        tc.tile_pool(name=f"sT{layer}", bufs=3) as stpool,
                        tc.tile_pool(name=f"ev{layer}", bufs=4) as evpool,
                        tc.tile_pool(name=f"pse{layer}", bufs=2, space="PSUM") as pse,
                        tc.tile_pool(name=f"pst{layer}", bufs=2, space="PSUM") as pst,
                        tc.tile_pool(name=f"pso{layer}", bufs=2, space="PSUM") as pso,
                    ):
                        PMAXB = sched["PMAXB"]
                        for pidx, pair in enumerate(sched["PAIRS"]):
                            t = pair["start"]
                            nb = pair["total"]
                            gt = gpool.tile([P, PMAXB * HC], FP16, tag="g")
                            gt3 = gt[:].rearrange("p (c e) -> p c e", e=HC)
                            if TRIAGE != "nogather":
                                for h, (cs, nbh) in enumerate(pair["parts"]):
                                    if nbh == 0:
                                        continue
                                    nc.gpsimd.dma_gather(
                                        gt3[:, cs - t:cs - t + nbh, :], tabs[h],
                                        idx_t[:, 8 * cs: 8 * (cs + nbh)],
                                        nbh * P, nbh * P, HC, elem_step=HC,
                                        single_packet=False,
                                        queue_num=(npart * pidx + h) % 4,
                                    )
                            if TRIAGE == "gather":
                                continue

                            # one-hot S per block; ad-expansion via PE
                            ps_e = pse.tile([P, PMAXB * HEADS], FP32, tag="pe")
                            s_tiles = {}
                            for g, slots in pair["groups"]:
                                for q in slots:
                                    S = spool.tile([P, P], FP16, tag="S")
                                    nc.vector.tensor_scalar(
                                        S[:], iota_t[:],
                                        dl_t[:, t + q: t + q + 1],
                                        None, mybir.AluOpType.is_equal)
                                    s_tiles[q] = S
                                    pSt = pst.tile([P, P], FP16, tag="pst")
                                    nc.tensor.transpose(pSt[:], S[:], ident_t[:])
                                    St = stpool.tile([P, P], FP16, tag="St")
                                    nc.scalar.copy(St[:], pSt[:])
                                    nc.tensor.matmul(
                                        ps_e[:, q * HEADS:(q + 1) * HEADS],
                                        lhsT=St[:],
                                        rhs=alde_sb[:, g * HEADS:(g + 1) * HEADS],
                                        start=True, stop=True)

                            # a_src-dots from gathered rows, batched per pair;
                            # the products live in the (dead) mz msg area
                            mz = mzpool.tile([P, PMAXB * W2COLS], FP16, tag="mz")
                            mz3 = mz[:].rearrange("p (c e) -> p c e", e=W2COLS)
                            asb = bass.AP(
                                asrcb_t[:].tensor, asrcb_t[:].offset,
                                [asrcb_t[:].ap[0], [0, nb], [1, HC]])
                            nc.vector.tensor_tensor(
                                out=mz3[:, 0:nb, 0:HC], in0=gt3[:, 0:nb, :],
                                in1=asb, op=mybir.AluOpType.mult)
                            ase = evpool.tile([P, PMAXB * HEADS], FP32, tag="ae")
                            nc.vector.tensor_reduce(
                                out=ase[:, 0:nb * HEADS],
                                in_=bass.AP(
                                    mz3.tensor, mz3.offset,
                                    [mz3.ap[0], [W2COLS, nb],
                                     [OUT_C, HEADS], [1, OUT_C]]),
                                axis=mybir.AxisListType.X,
                                op=mybir.AluOpType.add)

                            # ex = exp(lrelu(as + ad)), batched per pair;
                            # written into mz per-block tail columns (256:260)
                            exv = evpool.tile([P, PMAXB * HEADS], FP32, tag="ex")
                            nc.vector.tensor_tensor(
                                out=exv[:, 0:nb * HEADS],
                                in0=ase[:, 0:nb * HEADS],
                                in1=ps_e[:, 0:nb * HEADS],
                                op=mybir.AluOpType.add)
                            lrt = evpool.tile([P, PMAXB * HEADS], FP32, tag="lr")
                            nc.vector.tensor_scalar(
                                lrt[:, 0:nb * HEADS], exv[:, 0:nb * HEADS],
                                NEG_SLOPE, None, mybir.AluOpType.mult)
                            nc.vector.tensor_tensor(
                                out=exv[:, 0:nb * HEADS],
                                in0=exv[:, 0:nb * HEADS],
                                in1=lrt[:, 0:nb * HEADS],
                                op=mybir.AluOpType.max)
                            nc.scalar.activation(
                                mz3[:, 0:nb, HC:W2COLS],
                                exv[:].rearrange(
                                    "p (c e) -> p c e", e=HEADS)[:, 0:nb, :],
                                mybir.ActivationFunctionType.Exp)

                            # messages + segment-sum via PE, per group
                            for g, slots in pair["groups"]:
                                ps_out = pso.tile([P, W2COLS], FP32, tag="po")
                                for j, q in enumerate(slots):
                                    exs = bass.AP(
                                        mz3.tensor,
                                        mz3.offset + (q * W2COLS + HC),
                                        [mz3.ap[0], [1, HEADS], [0, OUT_C]])
                                    nc.vector.tensor_tensor(
                                        out=mz3[:, q, 0:HC], in0=gt3[:, q, :],
                                        in1=exs, op=mybir.AluOpType.mult)
                                    nc.tensor.matmul(
                                        ps_out[:],
                                        lhsT=s_tiles[q][:],
                                        rhs=mz3[:, q, 0:W2COLS],
                                        start=(j == 0),
                                        stop=(j == len(slots) - 1))
                                evac(g, ps_out, evpool, pst)

                # ---------- evac 1: f1 -> f1T, layer-2 transform, chunk AG ---
                with (
                    tc.tile_pool(name=f"p2a{rep}", bufs=3) as p2a,
                    tc.tile_pool(name=f"p2a_ps{rep}", bufs=2, space="PSUM") as p2a_ps,
                ):
                    def evac1(g, ps_out, evpool, pst):
                        rec = evpool.tile([P, HEADS], FP32, tag="rc")
                        nc.vector.tensor_scalar(
                            rec[:], ps_out[:, HC:W2COLS], 1e-16, None,
                            mybir.AluOpType.add)
                        nc.vector.reciprocal(rec[:], rec[:])
                        recb = bass.AP(
                            rec[:].tensor, rec[:].offset,
                            [rec[:].ap[0], [1, HEADS], [0, OUT_C]])
                        ftmp = evpool.tile([P, HC], FP16 if not add_b1 else FP32,
                                           tag="f1")
                        nc.vector.tensor_tensor(
                            out=ftmp[:], in0=ps_out[:, 0:HC], in1=recb,
                            op=mybir.AluOpType.mult)
                        if add_b1:
                            nc.vector.tensor_tensor(
                                out=ftmp[:], in0=ftmp[:], in1=b1_t[:],
                                op=mybir.AluOpType.add)
                        for k in range(2):
                            pft = pst.tile([P, P], FP16, tag="pst")
                            nc.tensor.transpose(
                                pft[:], ftmp[:, k * P:(k + 1) * P], ident_t[:])
                            nc.scalar.copy(
                                f1T_sb[:, g * HC + k * P: g * HC + (k + 1) * P],
                                pft[:])
                        # layer-2 node transform for this group
                        ps2 = p2a_ps.tile([P, W2COLS], FP32, tag="p2")
                        nc.tensor.matmul(
                            ps2[:], lhsT=f1T_sb[:, g * HC: g * HC + P],
                            rhs=w2a_t[:], start=True, stop=False)
                        nc.tensor.matmul(
                            ps2[:], lhsT=f1T_sb[:, g * HC + P: g * HC + 2 * P],
                            rhs=w2b_t[:], start=False, stop=True)
                        t2x = p2a.tile([P, HC], FP16, tag="t2x")
                        nc.scalar.copy(t2x[:], ps2[:, 0:HC])
                        nc.sync.dma_start(
                            out=t2own[g * P:(g + 1) * P, :], in_=t2x[:])
                        nc.vector.tensor_copy(
                            alde2_sb[:, g * HEADS:(g + 1) * HEADS],
                            ps2[:, HC:W2COLS])
                        agbounds = (*G2SPLITS, G)
                        if TRIAGE not in ("noag",) and g + 1 in agbounds:
                            i = agbounds.index(g + 1)
                            lo_g = 0 if i == 0 else agbounds[i - 1]
                            nc.gpsimd.collective_compute(
                                "AllGather",
                                mybir.AluOpType.bypass,
                                replica_groups=[list(range(N_CORES))],
                                ins=[t2own[lo_g * P:(g + 1) * P, :].opt()],
                                outs=[t2tiles[i][:].opt()],
                            )

                    if TRIAGE != "noedge":
                        edge_phase(f"1_{rep}", scheds[1],
                                   [table1[0:SPLIT, :], table1[SPLIT:NPAD, :]],
                                   idx1_t, dl1_t, alde1_sb, asrcb1_t, evac1)

                # ---------- edge phase 2 + final evacuation ----------
                def evac2(g, ps_out, evpool, pst):
                    rec = evpool.tile([P, HEADS], FP32, tag="rc2")
                    nc.vector.tensor_scalar(
                        rec[:], ps_out[:, HC:W2COLS], 1e-16, float(HEADS),
                        mybir.AluOpType.add, mybir.AluOpType.mult)
                    nc.vector.reciprocal(rec[:], rec[:])
                    recb = bass.AP(
                        rec[:].tensor, rec[:].offset,
                        [rec[:].ap[0], [1, HEADS], [0, OUT_C]])
                    tmp = evpool.tile([P, HC], FP32, tag="tm2")
                    nc.vector.tensor_tensor(
                        out=tmp[:], in0=ps_out[:, 0:HC], in1=recb,
                        op=mybir.AluOpType.mult)
                    tmpv = bass.AP(
                        tmp[:].tensor, tmp[:].offset,
                        [tmp[:].ap[0], [1, OUT_C], [OUT_C, HEADS]])
                    hsum = evpool.tile([P, OUT_C], FP32, tag="hs")
                    nc.vector.tensor_reduce(
                        out=hsum[:], in_=tmpv, axis=mybir.AxisListType.X,
                        op=mybir.AluOpType.add)
                    ob = evpool.tile([P, OUT_C], FP32, tag="ob")
                    nc.vector.tensor_tensor(
                        out=ob[:], in0=hsum[:],
                        in1=xch_sb[:, g * OUT_C:(g + 1) * OUT_C],
                        op=mybir.AluOpType.add)
                    nc.sync.dma_start(
                        out=out_d[g * P:(g + 1) * P, :], in_=ob[:])

                if TRIAGE != "noedge":
                    edge_phase(f"2_{rep}", scheds[2],
                               [tt[:] for tt in t2tiles],
                               idx2_t, dl2_t, alde2_sb, asrcb2_t, evac2)

    nc.compile()
    return nc


# ---------------------------------------------------------------------------
# Host-side input prep shared by kernel() and test.py
# ---------------------------------------------------------------------------

def make_weights(W1, a_dst1, Wfc, W2, a_dst2, a_src1, a_src2):
    w1r = np.asarray(W1, np.float32).reshape(IN_C, HEADS, OUT_C)
    w1_ad = np.einsum("khc,hc->kh", w1r, np.asarray(a_dst1, np.float32))
    w1ext = np.concatenate(
        [np.asarray(W1, np.float32), w1_ad, np.asarray(Wfc, np.float32)],
        axis=1).astype(np.float16)
    w2r = np.asarray(W2, np.float32).reshape(HC, HEADS, OUT_C)
    w2_ad = np.einsum("khc,hc->kh", w2r, np.asarray(a_dst2, np.float32))
    w2ext = np.concatenate(
        [np.asarray(W2, np.float32), w2_ad], axis=1).astype(np.float16)
    asrcb1 = np.broadcast_to(
        np.asarray(a_src1, np.float32).reshape(-1), (P, HC)).astype(np.float16)
    asrcb2 = np.broadcast_to(
        np.asarray(a_src2, np.float32).reshape(-1), (P, HC)).astype(np.float16)
    return w1ext, w2ext, asrcb1.copy(), asrcb2.copy()


def make_in_maps(x, edge_index, W1, a_src1, a_dst1, W2, a_src2, a_dst2,
                 Wfc, b1, scheds, coredata):
    x = np.asarray(x, np.float32)
    w1ext, w2ext, asrcb1, asrcb2 = make_weights(
        W1, a_dst1, Wfc, W2, a_dst2, a_src1, a_src2)
    xpadT = np.zeros((IN_C, NPAD), dtype=np.float16)
    xpadT[:, :N] = x.T.astype(np.float16)
    iota = np.broadcast_to(
        np.arange(P, dtype=np.float32), (P, P)).copy()
    ident = np.eye(P, dtype=np.float16)
    add_b1 = bool(np.any(np.asarray(b1) != 0))
    in_maps = []
    for m in range(N_CORES):
        im = dict(
            xTfull=xpadT,
            xTown=np.ascontiguousarray(xpadT[:, m * NPC:(m + 1) * NPC]),
            idx1=coredata[m]["idx1"], idx2=coredata[m]["idx2"],
            dl1=coredata[m]["dl1"], dl2=coredata[m]["dl2"],
            w1ext=w1ext, w2ext=w2ext, asrcb1=asrcb1, asrcb2=asrcb2,
            iota=iota, ident=ident,
        )
        if add_b1:
            im["b1rep"] = np.broadcast_to(
                np.asarray(b1, np.float32), (P, HC)).copy()
        in_maps.append(im)
    return in_maps, add_b1


# ---------------------------------------------------------------------------
# Entry point
# ---------------------------------------------------------------------------

def kernel(x, edge_index, W1, a_src1, a_dst1, b1, W2, a_src2, a_dst2, b2,
           Wfc, bfc):
    scheds, coredata = _preprocess(edge_index)
    in_maps, add_b1 = make_in_maps(
        x, edge_index, W1, a_src1, a_dst1, W2, a_src2, a_dst2, Wfc, b1,
        scheds, coredata)
    nc = _build_program(scheds, add_b1)
    res = run_bass_kernel_spmd(nc, in_maps, list(range(N_CORES)))
    outs = []
    for m in range(N_CORES):
        hi = min(N - m * NPC, NPC)
        outs.append(res.results[m]["out"][:hi])
    out = np.concatenate(outs, axis=0)
    out = out + (np.asarray(b2, np.float32) + np.asarray(bfc, np.float32))[None, :]
    return out.astype(np.float32)


# revision 20
# speedup vs baseline: 1.6220x; 1.6220x over previous
"""Trainium2 Bass kernel for a 2-layer GAT block (gnn_message_passing).

Strategy (8 NeuronCores, dst-node sharding, fp16 tables, dma_gather):
  - Nodes padded to 50176 = 8*6272; core m owns rows [6272m, 6272(m+1)).
    49 groups of 128 own dst nodes per core.
  - Layer-1 node transform is REPLICATED (x is a full input on every core):
    each core computes the whole table1[50176, 256] = fp16(x @ W1) in HBM.
    No collective for layer 1.  A small per-core pass computes a_dst1-dots
    and the residual x@Wfc for own nodes only.
  - Edge phase: edges sharded by dst; per 128-dst group the src rows are
    fetched with ONE dma_gather per (group, table-half) (int16 indices cap
    rows at 32768, so tables are split at row 25088).  Per 128-edge block a
    one-hot S matrix turns segment-sum and a_dst-expansion into PE matmuls;
    a_src-dots are recomputed on-chip from the gathered rows (keeps table
    rows at 512B).  ex = exp(leaky_relu(as+ad)) batched per group.
  - f1 evacuation transposes own f1 into SBUF; layer-2 node transform for
    own nodes feeds table2 chunks that are AllGathered (7 chunks of 7
    groups) OVERLAPPED with the remaining layer-1 edge work.
  - Edge phase 2 gathers from the AllGathered table2 (chunked row layout,
    indices precomputed on host), evacuates mean-over-heads + residual.
"""

import numpy as np

import concourse.bass as bass
import concourse.bacc as bacc
import concourse.mybir as mybir
import concourse.tile as tile
from concourse.bass_utils import run_bass_kernel_spmd

# Problem constants (hardcoded per harness contract)
N = 50000
E = 800000
IN_C = 128
OUT_C = 64
HEADS = 4
NEG_SLOPE = 0.2
N_CORES = 8

P = 128
NPC = 6272                 # own nodes per core (padded); 8*6272 = 50176
G = NPC // P               # 49 own groups per core
NPAD = N_CORES * NPC       # 50176
GALL = NPAD // P           # 392 groups in the replicated layer-1 transform
SPLIT = NPAD // 2          # 25088: table half split (int16 gather indices)
# table2 is split into 3 sub-tables (AllGathered as soon as their groups
# are evacuated, overlapping remaining edge-1 work).  Own-group ranges:
G2SPLITS = (16, 32)        # sub-table a: groups [0,16), b: [16,32), c: [32,49)
T2SIZES = (N_CORES * 16 * P, N_CORES * 16 * P, N_CORES * 17 * P)
T2BASES = (0, T2SIZES[0], T2SIZES[0] + T2SIZES[1])
HC = HEADS * OUT_C         # 256
TROW = 384                 # table row stride (768B, %256; used: 264)
TUSED = HC + HEADS         # 260 cols of each table row carry data
RCOLS = HC + 2 * HEADS     # 264: segsum rhs = msg 256 | as 4 | ex 4
W1COLS = HC + 2 * HEADS + OUT_C  # 328: W1 | a_src-dot | a_dst-dot | Wfc
W2COLS = HC + 2 * HEADS          # 264: W2 | a_src-dot | a_dst-dot

FP32 = mybir.dt.float32
FP16 = mybir.dt.float16
I16 = mybir.dt.int16

# timing-triage mode (set by triage.py): None | "noedge" | "gather" |
# "noag" | "nogather" | "nosblock"
TRIAGE = None


def _ceil_div(a, b):
    return (a + b - 1) // b


# ---------------------------------------------------------------------------
# Host-side preprocessing
# ---------------------------------------------------------------------------

def _row2_of_src(src):
    """Row of node `src` in the three-part table2 layout ([m, gg, r]-major
    within each sub-table; sub-table bases offset the combined index)."""
    m = src // NPC
    loc = src % NPC
    gg = loc >> 7
    r = loc & 127
    a = 2048 * m + 128 * gg + r
    b = T2BASES[1] + 2048 * m + 128 * (gg - 16) + r
    c = T2BASES[2] + 2176 * m + 128 * (gg - 32) + r
    return np.where(gg < 16, a, np.where(gg < 32, b, c))


def _preprocess(edge_index):
    """Sort/shard/pad edges; per-layer gather indices + dst-local arrays with
    a block schedule that is uniform across cores (SPMD: one program)."""
    src = np.asarray(edge_index[0], dtype=np.int64)
    dst = np.asarray(edge_index[1], dtype=np.int64)
    loops = np.arange(N, dtype=np.int64)
    src = np.concatenate([src, loops]).astype(np.int64)
    dst = np.concatenate([dst, loops]).astype(np.int64)

    core = dst // NPC
    # table1 is partition-major: node (g, r) = (src>>7, src&127) sits at
    # row r*GALL + g, making phase-A table writes contiguous per partition.
    row1 = (src & 127) * GALL + (src >> 7)
    row2 = _row2_of_src(src)

    LBASES = {1: [0, SPLIT], 2: list(T2BASES)}

    percore = []   # per core: dict(layer -> (rows, key, dloc))
    cnts = {1: [], 2: []}   # per core: [G, nparts] counts
    for m in range(N_CORES):
        mask = core == m
        cs = src[mask]
        r1 = row1[mask]
        cd = dst[mask] - m * NPC
        gg = cd >> 7
        dl = cd & 127
        r2 = row2[mask]
        layers = {}
        for l, rows in ((1, r1), (2, r2)):
            bases = LBASES[l]
            npart = len(bases)
            part = np.searchsorted(bases[1:], rows, side="right")
            o = np.lexsort((rows, part, gg))
            lr = rows[o]
            lp = part[o]
            lg = gg[o]
            ld = dl[o]
            cnt = np.zeros((G, npart), dtype=np.int64)
            np.add.at(cnt, (lg, lp), 1)
            layers[l] = (lr, lg * npart + lp, ld)
            cnts[l].append(cnt)
        percore.append(layers)

    scheds = {}
    for l in (1, 2):
        allc = np.stack(cnts[l])                  # [cores, G, nparts]
        nbp = np.maximum(_ceil_div(allc, P).max(axis=0), 1)  # [G, nparts]
        btot = int(nbp.sum())
        scheds[l] = dict(NBP=nbp, BTOT=btot, MAXB=int(nbp.sum(axis=1).max()),
                         BASES=LBASES[l])

    # Pair-merged block schedule: groups are processed in pairs (2k, 2k+1);
    # within a pair, blocks are ordered part-major ((g0,h0),(g1,h0),(g0,h1),
    # ...) so ONE dma_gather per (pair, part) covers both groups.  PAIRS[k]
    # holds per-part (col_start, nblocks) and per-group block-slot lists.
    for l in (1, 2):
        nbp = scheds[l]["NBP"]
        npart = nbp.shape[1]
        pairs = []
        t = 0
        for k in range(0, G, 2):
            gs = [k] if k + 1 >= G else [k, k + 1]
            parts = []
            slots = {g: [] for g in gs}
            start = t
            for h in range(npart):
                cs = t
                for g in gs:
                    nbh = int(nbp[g, h])
                    slots[g].extend(range(t - start, t - start + nbh))
                    t += nbh
                parts.append((cs, t - cs))
            pairs.append(dict(start=start, parts=parts,
                              groups=[(g, slots[g]) for g in gs],
                              total=t - start))
        scheds[l]["PAIRS"] = pairs
        scheds[l]["PMAXB"] = max(p["total"] for p in pairs)
        assert t == scheds[l]["BTOT"]

    # per-core padded arrays (same pair-merged order)
    coredata = []
    for m in range(N_CORES):
        out = {}
        for l in (1, 2):
            nbp = scheds[l]["NBP"]
            bases = scheds[l]["BASES"]
            npart = len(bases)
            btot = scheds[l]["BTOT"]
            rows, key, dloc = percore[m][l]
            order_bounds = np.searchsorted(key, np.arange(npart * G + 1))
            idxw = np.zeros((128, 8 * btot), dtype=np.int16)
            dl_arr = np.full((128, btot), -1.0, dtype=np.float32)
            t = 0
            for k in range(0, G, 2):
                gs = [k] if k + 1 >= G else [k, k + 1]
                for h in range(npart):
                    for g in gs:
                        nbh = int(nbp[g, h])
                        a, b = (order_bounds[npart * g + h],
                                order_bounds[npart * g + h + 1])
                        ne = b - a
                        npadd = nbh * P - ne
                        assert npadd >= 0
                        rr = np.concatenate([
                            rows[a:b] - bases[h],
                            np.zeros(npadd, np.int64)]).astype(np.int16)
                        dd = np.concatenate([
                            dloc[a:b].astype(np.float32),
                            np.full(npadd, -1.0, np.float32)])
                        nn = nbh * P
                        iw = np.zeros((16, nn // 16), np.int16)
                        iw[np.arange(nn) % 16, np.arange(nn) // 16] = rr
                        idxw[:, 8 * t: 8 * (t + nbh)] = np.tile(iw, (8, 1))
                        dl_arr[np.arange(nn) % 128,
                               t + np.arange(nn) // 128] = dd
                        t += nbh
            assert t == btot
            out[f"idx{l}"] = idxw
            out[f"dl{l}"] = dl_arr
        coredata.append(out)
    return scheds, coredata


# ---------------------------------------------------------------------------
# Device program
# ---------------------------------------------------------------------------

def _build_program(scheds, add_b1, reps=1):
    nc = bacc.Bacc(
        "TRN2",
        target_bir_lowering=False,
        debug=False,
        enable_asserts=False,
        num_devices=N_CORES,
        num_swdge_queues=4,
    )

    B1, B2 = scheds[1]["BTOT"], scheds[2]["BTOT"]

    # ---- I/O ----
    xTfull_d = nc.dram_tensor("xTfull", [IN_C, NPAD], FP16, kind="ExternalInput")
    xTown_d = nc.dram_tensor("xTown", [IN_C, NPC], FP16, kind="ExternalInput")
    idx1_d = nc.dram_tensor("idx1", [128, 8 * B1], I16, kind="ExternalInput")
    idx2_d = nc.dram_tensor("idx2", [128, 8 * B2], I16, kind="ExternalInput")
    dl1_d = nc.dram_tensor("dl1", [128, B1], FP32, kind="ExternalInput")
    dl2_d = nc.dram_tensor("dl2", [128, B2], FP32, kind="ExternalInput")
    w1ext_d = nc.dram_tensor("w1ext", [IN_C, W1COLS], FP16, kind="ExternalInput")
    w2ext_d = nc.dram_tensor("w2ext", [HC, W2COLS], FP16, kind="ExternalInput")
    iota_d = nc.dram_tensor("iota", [P, P], FP32, kind="ExternalInput")
    ident_d = nc.dram_tensor("ident", [P, P], FP16, kind="ExternalInput")
    if add_b1:
        b1rep_d = nc.dram_tensor("b1rep", [P, HC], FP32, kind="ExternalInput")
    out_d = nc.dram_tensor("out", [NPC, OUT_C], FP32, kind="ExternalOutput")

    with tile.TileContext(nc) as tc:
        with (
            tc.tile_pool(name="const", bufs=1) as cpool,
            tc.tile_pool(name="dram", bufs=1, space="DRAM") as dpool,
        ):
            iota_t = cpool.tile([P, P], FP32)
            nc.sync.dma_start(out=iota_t[:], in_=iota_d[:])
            ident_t = cpool.tile([P, P], FP16)
            nc.sync.dma_start(out=ident_t[:], in_=ident_d[:])
            w1_t = cpool.tile([IN_C, W1COLS], FP16)
            nc.sync.dma_start(out=w1_t[:], in_=w1ext_d[:])
            w2a_t = cpool.tile([P, W2COLS], FP16)
            nc.sync.dma_start(out=w2a_t[:], in_=w2ext_d[0:P, :])
            w2b_t = cpool.tile([P, W2COLS], FP16)
            nc.sync.dma_start(out=w2b_t[:], in_=w2ext_d[P: 2 * P, :])
            idx1_t = cpool.tile([128, 8 * B1], I16)
            nc.sync.dma_start(out=idx1_t[:], in_=idx1_d[:])
            idx2_t = cpool.tile([128, 8 * B2], I16)
            nc.sync.dma_start(out=idx2_t[:], in_=idx2_d[:])
            dl1_t = cpool.tile([128, B1], FP32)
            nc.sync.dma_start(out=dl1_t[:], in_=dl1_d[:])
            dl2_t = cpool.tile([128, B2], FP32)
            nc.sync.dma_start(out=dl2_t[:], in_=dl2_d[:])
            if add_b1:
                b1_t = cpool.tile([P, HC], FP32)
                nc.sync.dma_start(out=b1_t[:], in_=b1rep_d[:])

            for rep in range(reps):
              with tc.tile_pool(name=f"state{rep}", bufs=1) as statepool:
                table1 = dpool.tile([NPAD, TROW], FP16, tag=f"t1_{rep}",
                                    name=f"table1_{rep}")
                t2own = dpool.tile([NPC, TROW], FP16, tag=f"t2o_{rep}",
                                   name=f"t2own_{rep}")
                t2tiles = [
                    dpool.tile([T2SIZES[i], TROW], FP16, addr_space="Shared",
                               tag=f"t2{i}_{rep}", name=f"table2{i}_{rep}")
                    for i in range(3)
                ]

                alde1_sb = statepool.tile([P, G * HEADS], FP16, tag="ad1")
                alde2_sb = statepool.tile([P, G * HEADS], FP16, tag="ad2")
                xch_sb = statepool.tile([P, G * OUT_C], FP32, tag="xch")
                f1T_sb = statepool.tile([P, G * HC], FP16, tag="f1T")

                # ---------- Phase A-own: a_dst1-dots + residual (own nodes) --
                with (
                    tc.tile_pool(name=f"po{rep}", bufs=3) as po,
                    tc.tile_pool(name=f"po_ps{rep}", bufs=2, space="PSUM") as po_ps,
                ):
                    xo = po.tile([IN_C, NPC], FP16, tag="xo")
                    nc.sync.dma_start(out=xo[:], in_=xTown_d[:])
                    for g in range(G):
                        ps = po_ps.tile([P, HEADS + OUT_C], FP32, tag="ps")
                        nc.tensor.matmul(
                            ps[:], lhsT=xo[:, g * P:(g + 1) * P],
                            rhs=w1_t[:, TUSED:W1COLS],
                            start=True, stop=True)
                        nc.vector.tensor_copy(
                            alde1_sb[:, g * HEADS:(g + 1) * HEADS],
                            ps[:, 0:HEADS])
                        nc.vector.tensor_copy(
                            xch_sb[:, g * OUT_C:(g + 1) * OUT_C],
                            ps[:, HEADS:HEADS + OUT_C])

                # ---------- Phase A-full: replicated layer-1 transform -------
                # super-groups of SG groups: one big x read + one big table
                # write per super-group (batched DMA).
                SG = 8
                with (
                    tc.tile_pool(name=f"pa{rep}", bufs=2) as pa,
                    tc.tile_pool(name=f"pa_ps{rep}", bufs=4, space="PSUM") as pa_ps,
                ):
                    for g0 in range(0, GALL, SG):
                        ng = min(SG, GALL - g0)
                        xf = pa.tile([IN_C, SG * P], FP16, tag="xf")
                        nc.sync.dma_start(
                            out=xf[:, 0:ng * P],
                            in_=xTfull_d[:, g0 * P:(g0 + ng) * P])
                        tx = pa.tile([P, SG * TUSED], FP16, tag="tx")
                        for k in range(ng):
                            ph = pa_ps.tile([P, TUSED], FP32, tag="ph")
                            nc.tensor.matmul(
                                ph[:], lhsT=xf[:, k * P:(k + 1) * P],
                                rhs=w1_t[:, 0:TUSED], start=True, stop=True)
                            nc.scalar.copy(
                                tx[:, k * TUSED:(k + 1) * TUSED], ph[:])
                        outap = bass.AP(
                            table1[:].tensor,
                            table1[:].offset + g0 * TROW,
                            [[GALL * TROW, P], [TROW, ng], [1, TUSED]])
                        nc.sync.dma_start(out=outap, in_=tx[:, 0:ng * TUSED])

                # ---------- Edge phase ----------
                def edge_phase(layer, sched, tabs, idx_t, dl_t,
                               alde_sb, evac):
                    NBP = sched["NBP"]
                    npart = NBP.shape[1]
                    PMAXB = sched["PMAXB"]
                    with (
                        tc.tile_pool(name=f"gt{layer}", bufs=2) as gpool,
                        tc.tile_pool(name=f"sA{layer}", bufs=2) as spool,
                        tc.tile_pool(name=f"sT{layer}", bufs=3) as stpool,
                        tc.tile_pool(name=f"ev{layer}", bufs=4) as evpool,
                        tc.tile_pool(name=f"pse{layer}", bufs=2, space="PSUM") as pse,
                        tc.tile_pool(name=f"pst{layer}", bufs=2, space="PSUM") as pst,
                        tc.tile_pool(name=f"pso{layer}", bufs=2, space="PSUM") as pso,
                    ):
                        for pidx, pair in enumerate(sched["PAIRS"]):
                            t = pair["start"]
                            nb = pair["total"]
                            gt = gpool.tile([P, PMAXB * TROW], FP16, tag="g")
                            gt3 = gt[:].rearrange("p (c e) -> p c e", e=TROW)
                            if TRIAGE != "nogather":
                                for h, (cs, nbh) in enumerate(pair["parts"]):
                                    if nbh == 0:
                                        continue
                                    nc.gpsimd.dma_gather(
                                        gt3[:, cs - t:cs - t + nbh, 0:TROW],
                                        tabs[h],
                                        idx_t[:, 8 * cs: 8 * (cs + nbh)],
                                        nbh * P, nbh * P, TROW, elem_step=TROW,
                                        single_packet=False,
                                        queue_num=(npart * pidx + h) % 4,
                                    )
                            if TRIAGE == "gather":
                                continue

                            # all one-hot S matrices of the pair in ONE op:
                            # S_all[p, q*128+d] = (iota[d] == dl[p, t+q])
                            s_all = spool.tile([P, PMAXB * P], FP16, tag="S")
                            sa3 = s_all[:].rearrange("p (c e) -> p c e", e=P)
                            iob = bass.AP(
                                iota_t[:].tensor, iota_t[:].offset,
                                [iota_t[:].ap[0], [0, nb], [1, P]])
                            dlb = bass.AP(
                                dl_t[:].tensor, dl_t[:].offset + t,
                                [dl_t[:].ap[0], [1, nb], [0, P]])
                            nc.vector.tensor_tensor(
                                out=sa3[:, 0:nb, :], in0=iob, in1=dlb,
                                op=mybir.AluOpType.is_equal)

                            # ad-expansion via PE (transpose S, S^T @ alde)
                            ps_e = pse.tile([P, PMAXB * HEADS], FP32, tag="pe")
                            for g, slots in pair["groups"]:
                                for q in slots:
                                    pSt = pst.tile([P, P], FP16, tag="pst")
                                    nc.tensor.transpose(
                                        pSt[:], sa3[:, q, :], ident_t[:])
                                    St = stpool.tile([P, P], FP16, tag="St")
                                    nc.scalar.copy(St[:], pSt[:])
                                    nc.tensor.matmul(
                                        ps_e[:, q * HEADS:(q + 1) * HEADS],
                                        lhsT=St[:],
                                        rhs=alde_sb[:, g * HEADS:(g + 1) * HEADS],
                                        start=True, stop=True)

                            # ex = exp(lrelu(as + ad)) batched per pair,
                            # written into gt row cols 260:264
                            exv = evpool.tile([P, PMAXB * HEADS], FP32, tag="ex")
                            asv = bass.AP(
                                gt3.tensor, gt3.offset + HC,
                                [gt3.ap[0], [TROW, nb], [1, HEADS]])
                            nc.vector.tensor_tensor(
                                out=exv[:, 0:nb * HEADS],
                                in0=asv,
                                in1=ps_e[:, 0:nb * HEADS],
                                op=mybir.AluOpType.add)
                            lrt = evpool.tile([P, PMAXB * HEADS], FP32, tag="lr")
                            nc.vector.tensor_scalar(
                                lrt[:, 0:nb * HEADS], exv[:, 0:nb * HEADS],
                                NEG_SLOPE, None, mybir.AluOpType.mult)
                            nc.vector.tensor_tensor(
                                out=exv[:, 0:nb * HEADS],
                                in0=exv[:, 0:nb * HEADS],
                                in1=lrt[:, 0:nb * HEADS],
                                op=mybir.AluOpType.max)
                            exq = bass.AP(
                                gt3.tensor, gt3.offset + TUSED,
                                [gt3.ap[0], [TROW, nb], [1, HEADS]])
                            nc.scalar.activation(
                                exq,
                                exv[:].rearrange(
                                    "p (c e) -> p c e", e=HEADS)[:, 0:nb, :],
                                mybir.ActivationFunctionType.Exp)

                            # message scaling, in-place on gt, ONE op per pair
                            if TRIAGE != "nomsg":
                                mout = bass.AP(
                                    gt3.tensor, gt3.offset,
                                    [gt3.ap[0], [TROW, nb],
                                     [OUT_C, HEADS], [1, OUT_C]])
                                mex = bass.AP(
                                    gt3.tensor, gt3.offset + TUSED,
                                    [gt3.ap[0], [TROW, nb],
                                     [1, HEADS], [0, OUT_C]])
                                nc.vector.tensor_tensor(
                                    out=mout, in0=mout, in1=mex,
                                    op=mybir.AluOpType.mult)

                            # segment-sum via PE, per group
                            for g, slots in pair["groups"]:
                                ps_out = pso.tile([P, RCOLS], FP32, tag="po")
                                for j, q in enumerate(slots):
                                    nc.tensor.matmul(
                                        ps_out[:],
                                        lhsT=sa3[:, q, :],
                                        rhs=gt3[:, q, 0:RCOLS],
                                        start=(j == 0),
                                        stop=(j == len(slots) - 1))
                                evac(g, ps_out, evpool, pst)

                # ---------- evac 1: f1 -> f1T, layer-2 transform, chunk AG ---
                with (
                    tc.tile_pool(name=f"p2a{rep}", bufs=3) as p2a,
                    tc.tile_pool(name=f"p2a_ps{rep}", bufs=2, space="PSUM") as p2a_ps,
                ):
                    def evac1(g, ps_out, evpool, pst):
                        rec = evpool.tile([P, HEADS], FP32, tag="rc")
                        nc.vector.tensor_scalar(
                            rec[:], ps_out[:, TUSED:RCOLS], 1e-16, None,
                            mybir.AluOpType.add)
                        nc.vector.reciprocal(rec[:], rec[:])
                        recb = bass.AP(
                            rec[:].tensor, rec[:].offset,
                            [rec[:].ap[0], [1, HEADS], [0, OUT_C]])
                        ftmp = evpool.tile([P, HC], FP16 if not add_b1 else FP32,
                                           tag="f1")
                        nc.vector.tensor_tensor(
                            out=ftmp[:], in0=ps_out[:, 0:HC], in1=recb,
                            op=mybir.AluOpType.mult)
                        if add_b1:
                            nc.vector.tensor_tensor(
                                out=ftmp[:], in0=ftmp[:], in1=b1_t[:],
                                op=mybir.AluOpType.add)
                        for k in range(2):
                            pft = pst.tile([P, P], FP16, tag="pst")
                            nc.tensor.transpose(
                                pft[:], ftmp[:, k * P:(k + 1) * P], ident_t[:])
                            nc.scalar.copy(
                                f1T_sb[:, g * HC + k * P: g * HC + (k + 1) * P],
                                pft[:])
                        # layer-2 node transform for this group
                        ps2 = p2a_ps.tile([P, W2COLS], FP32, tag="p2")
                        nc.tensor.matmul(
                            ps2[:], lhsT=f1T_sb[:, g * HC: g * HC + P],
                            rhs=w2a_t[:], start=True, stop=False)
                        nc.tensor.matmul(
                            ps2[:], lhsT=f1T_sb[:, g * HC + P: g * HC + 2 * P],
                            rhs=w2b_t[:], start=False, stop=True)
                        t2x = p2a.tile([P, TUSED], FP16, tag="t2x")
                        nc.scalar.copy(t2x[:], ps2[:, 0:TUSED])
                        nc.sync.dma_start(
                            out=t2own[g * P:(g + 1) * P, 0:TUSED], in_=t2x[:])
                        nc.vector.tensor_copy(
                            alde2_sb[:, g * HEADS:(g + 1) * HEADS],
                            ps2[:, TUSED:W2COLS])
                        agbounds = (*G2SPLITS, G)
                        if TRIAGE not in ("noag",) and g + 1 in agbounds:
                            i = agbounds.index(g + 1)
                            lo_g = 0 if i == 0 else agbounds[i - 1]
                            nc.gpsimd.collective_compute(
                                "AllGather",
                                mybir.AluOpType.bypass,
                                replica_groups=[list(range(N_CORES))],
                                ins=[t2own[lo_g * P:(g + 1) * P, :].opt()],
                                outs=[t2tiles[i][:].opt()],
                            )

                    if TRIAGE != "noedge":
                        edge_phase(f"1_{rep}", scheds[1],
                                   [table1[0:SPLIT, :], table1[SPLIT:NPAD, :]],
                                   idx1_t, dl1_t, alde1_sb, evac1)

                # ---------- edge phase 2 + final evacuation ----------
                def evac2(g, ps_out, evpool, pst):
                    rec = evpool.tile([P, HEADS], FP32, tag="rc2")
                    nc.vector.tensor_scalar(
                        rec[:], ps_out[:, TUSED:RCOLS], 1e-16, float(HEADS),
                        mybir.AluOpType.add, mybir.AluOpType.mult)
                    nc.vector.reciprocal(rec[:], rec[:])
                    recb = bass.AP(
                        rec[:].tensor, rec[:].offset,
                        [rec[:].ap[0], [1, HEADS], [0, OUT_C]])
                    tmp = evpool.tile([P, HC], FP32, tag="tm2")
                    nc.vector.tensor_tensor(
                        out=tmp[:], in0=ps_out[:, 0:HC], in1=recb,
                        op=mybir.AluOpType.mult)
                    tmpv = bass.AP(
                        tmp[:].tensor, tmp[:].offset,
                        [tmp[:].ap[0], [1, OUT_C], [OUT_C, HEADS]])
                    hsum = evpool.tile([P, OUT_C], FP32, tag="hs")
                    nc.vector.tensor_reduce(
                        out=hsum[:], in_=tmpv, axis=mybir.AxisListType.X,
                        op=mybir.AluOpType.add)
                    ob = evpool.tile([P, OUT_C], FP32, tag="ob")
                    nc.vector.tensor_tensor(
                        out=ob[:], in0=hsum[:],
                        in1=xch_sb[:, g * OUT_C:(g + 1) * OUT_C],
                        op=mybir.AluOpType.add)
                    nc.sync.dma_start(
                        out=out_d[g * P:(g + 1) * P, :], in_=ob[:])

                if TRIAGE != "noedge":
                    edge_phase(f"2_{rep}", scheds[2],
                               [tt[:] for tt in t2tiles],
                               idx2_t, dl2_t, alde2_sb, evac2)

    nc.compile()
    return nc


# ---------------------------------------------------------------------------
# Host-side input prep shared by kernel() and test.py
# ---------------------------------------------------------------------------

def make_weights(W1, a_dst1, Wfc, W2, a_dst2, a_src1, a_src2):
    w1r = np.asarray(W1, np.float32).reshape(IN_C, HEADS, OUT_C)
    w1_as = np.einsum("khc,hc->kh", w1r, np.asarray(a_src1, np.float32))
    w1_ad = np.einsum("khc,hc->kh", w1r, np.asarray(a_dst1, np.float32))
    w1ext = np.concatenate(
        [np.asarray(W1, np.float32), w1_as, w1_ad,
         np.asarray(Wfc, np.float32)], axis=1).astype(np.float16)
    w2r = np.asarray(W2, np.float32).reshape(HC, HEADS, OUT_C)
    w2_as = np.einsum("khc,hc->kh", w2r, np.asarray(a_src2, np.float32))
    w2_ad = np.einsum("khc,hc->kh", w2r, np.asarray(a_dst2, np.float32))
    w2ext = np.concatenate(
        [np.asarray(W2, np.float32), w2_as, w2_ad], axis=1).astype(np.float16)
    return w1ext, w2ext


def make_in_maps(x, edge_index, W1, a_src1, a_dst1, W2, a_src2, a_dst2,
                 Wfc, b1, scheds, coredata):
    x = np.asarray(x, np.float32)
    w1ext, w2ext = make_weights(
        W1, a_dst1, Wfc, W2, a_dst2, a_src1, a_src2)
    xpadT = np.zeros((IN_C, NPAD), dtype=np.float16)
    xpadT[:, :N] = x.T.astype(np.float16)
    iota = np.broadcast_to(
        np.arange(P, dtype=np.float32), (P, P)).copy()
    ident = np.eye(P, dtype=np.float16)
    add_b1 = bool(np.any(np.asarray(b1) != 0))
    in_maps = []
    for m in range(N_CORES):
        im = dict(
            xTfull=xpadT,
            xTown=np.ascontiguousarray(xpadT[:, m * NPC:(m + 1) * NPC]),
            idx1=coredata[m]["idx1"], idx2=coredata[m]["idx2"],
            dl1=coredata[m]["dl1"], dl2=coredata[m]["dl2"],
            w1ext=w1ext, w2ext=w2ext,
            iota=iota, ident=ident,
        )
        if add_b1:
            im["b1rep"] = np.broadcast_to(
                np.asarray(b1, np.float32), (P, HC)).copy()
        in_maps.append(im)
    return in_maps, add_b1


# ---------------------------------------------------------------------------
# Entry point
# ---------------------------------------------------------------------------

def kernel(x, edge_index, W1, a_src1, a_dst1, b1, W2, a_src2, a_dst2, b2,
           Wfc, bfc):
    scheds, coredata = _preprocess(edge_index)
    in_maps, add_b1 = make_in_maps(
        x, edge_index, W1, a_src1, a_dst1, W2, a_src2, a_dst2, Wfc, b1,
        scheds, coredata)
    nc = _build_program(scheds, add_b1)
    res = run_bass_kernel_spmd(nc, in_maps, list(range(N_CORES)))
    outs = []
    for m in range(N_CORES):
        hi = min(N - m * NPC, NPC)
        outs.append(res.results[m]["out"][:hi])
    out = np.concatenate(outs, axis=0)
    out = out + (np.asarray(b2, np.float32) + np.asarray(bfc, np.float32))[None, :]
    return out.astype(np.float32)
